# revision 6
# baseline (speedup 1.0000x reference)
"""Causal multi-head attention block (B=4, S=1024, E=1024, H=16, D=64) on 8 TRN2 cores.

Sharding: data-parallel over batch (4) x tensor-parallel over heads (2 groups of 8).
Core i handles batch i//2, head-group i%2. Each core computes its partial output
projection (row-parallel W_proj); the host sums the two TP partials per batch and
applies the (exact) bias corrections.

Device-side math per core (bf16 compute, f32 accumulate):
  qT = (Wq_g)^T x^T + bq_g          [512, 1024]  (head-major rows h*64+d)
  kT = (Wk_g)^T x^T + bk_g          [512, 1024]
  v  = x Wv_g                       [1024, 512]
  For each head h: PT[sk, sq] = exp((kT_h^T qT_h)/8) * causal_mask (lower blocks only)
  PV runs head-PAIRS as concurrent column-tiled matmuls (head 2p -> PSUM rows 0-63,
  head 2p+1 -> rows 64-127); softmax denominators come from concurrent M=1
  ones-matmuls; a K=2 selector matmul broadcasts the per-head denominator row
  across the pair's 128 partitions, one reciprocal + one multiply normalizes.
  out_partial = o2T^T Wp_g          [1024, 1024]  (bf16 to DRAM)
Host: out[b] = out_partial[2b] + out_partial[2b+1] + (bv_0 Wp_0 + bv_1 Wp_1 + b_proj)
(the v-bias term is exact because softmax rows sum to 1).
"""

import numpy as np
import ml_dtypes

import concourse.bass as bass
import concourse.tile as tile
from concourse import bacc, mybir
from concourse.bass_utils import run_bass_kernel_spmd
from concourse.masks import make_upper_triangular

BF16 = mybir.dt.bfloat16
F32 = mybir.dt.float32

B, S, E = 4, 1024, 1024
H_TOT, D = 16, 64
NCORES = 8
HL = 8            # heads per core
JL = HL * D       # 512 local qkv dim
P = 128
ET = E // P       # 8 k-tiles over embed dim
JT = JL // P      # 4 partition-tiles over local qkv dim
EXP = mybir.ActivationFunctionType.Exp

_NC_CACHE = None


def build_nc():
    nc = bacc.Bacc()

    xT = nc.declare_dram_parameter("xT", [E, S], BF16, isOutput=False)
    wq = nc.declare_dram_parameter("wq", [E, JL], BF16, isOutput=False)
    wk = nc.declare_dram_parameter("wk", [E, JL], BF16, isOutput=False)
    wv = nc.declare_dram_parameter("wv", [E, JL], BF16, isOutput=False)
    wp = nc.declare_dram_parameter("wp", [JL, E], BF16, isOutput=False)
    bq = nc.declare_dram_parameter("bq", [P, JT], F32, isOutput=False)
    bk = nc.declare_dram_parameter("bk", [P, JT], F32, isOutput=False)
    out = nc.declare_dram_parameter("out", [S, E], BF16, isOutput=True)

    with tile.TileContext(nc) as tc:
        with (
            tc.tile_pool(name="singles", bufs=1) as singles,
            tc.tile_pool(name="pt", bufs=6) as pt_pool,
            tc.tile_pool(name="den", bufs=2) as den_pool,
            tc.tile_pool(name="bc", bufs=2) as bc_pool,
            tc.tile_pool(name="outst", bufs=2) as out_pool,
            tc.tile_pool(name="ps_l", bufs=2, space="PSUM") as ps_l,
            tc.tile_pool(name="ps_mm", bufs=2, space="PSUM") as ps_mm,
            tc.tile_pool(name="ps_o", bufs=2, space="PSUM") as ps_o,
        ):
            # ---- static inputs -> SBUF ----
            xT_sb = singles.tile([P, ET, S], BF16)
            wq_sb = singles.tile([P, ET, JL], BF16)
            wk_sb = singles.tile([P, ET, JL], BF16)
            wv_sb = singles.tile([P, ET, JL], BF16)
            wp_sb = singles.tile([P, JT, E], BF16)
            bq_sb = singles.tile([P, JT], F32)
            bk_sb = singles.tile([P, JT], F32)
            xT_r = xT[:, :].rearrange("(o p) s -> p o s", p=P)
            wq_r = wq[:, :].rearrange("(o p) j -> p o j", p=P)
            wk_r = wk[:, :].rearrange("(o p) j -> p o j", p=P)
            wv_r = wv[:, :].rearrange("(o p) j -> p o j", p=P)
            wp_r = wp[:, :].rearrange("(o p) e -> p o e", p=P)

            # x streams per-ktile on the sync queue; the first q/k chains
            # (jt0, kt-outer) consume tiles as they land
            for kt in range(ET):
                nc.sync.dma_start(out=xT_sb[:, kt], in_=xT_r[:, kt])
            # scalar (HWDGE) queue: jt0 slices of wq/wk first so the boot
            # chains start as soon as x tile 0 lands
            nc.scalar.dma_start(out=wq_sb[:, :, 0:P], in_=wq_r[:, :, 0:P])
            nc.scalar.dma_start(out=wk_sb[:, :, 0:P], in_=wk_r[:, :, 0:P])
            nc.scalar.dma_start(out=wq_sb[:, :, P:JL], in_=wq_r[:, :, P:JL])
            nc.scalar.dma_start(out=wk_sb[:, :, P:JL], in_=wk_r[:, :, P:JL])
            # gpsimd (SWDGE) queue: biases, then wv (needed ~15us), then wp
            nc.gpsimd.dma_start(out=bq_sb[:], in_=bq[:, :])
            nc.gpsimd.dma_start(out=bk_sb[:], in_=bk[:, :])
            for c in range(0, ET, 4):
                nc.gpsimd.dma_start(out=wv_sb[:, c:c + 4], in_=wv_r[:, c:c + 4])
            for c in range(0, JT, 2):
                nc.gpsimd.dma_start(out=wp_sb[:, c:c + 2], in_=wp_r[:, c:c + 2])

            # pre-trigger the exp ACT table load (~2.7us) during the DMA wait
            warm_in = singles.tile([1, 1], F32)
            warm_out = singles.tile([1, 1], F32)
            nc.vector.memset(warm_in[:, :], 0.0)
            nc.scalar.activation(out=warm_out[:, :], in_=warm_in[:, :], func=EXP)

            # causal keep-mask for diagonal PT blocks: 1 where sq >= sk else 0
            mask_sb = singles.tile([P, P], BF16)
            make_upper_triangular(nc, mask_sb[:], val=1.0, diag=True)

            # ones column (denominator matmuls) + pair-selector for the
            # K=2 broadcast matmul: row 0 -> partitions 0-63, row 1 -> 64-127
            ones_sb = singles.tile([P, 1], BF16)
            nc.vector.memset(ones_sb[:, :], 1.0)
            ones_row = singles.tile([1, 64], BF16)
            nc.vector.memset(ones_row[:, :], 1.0)

            qT_sb = singles.tile([P, JT, S], BF16)   # row j = h*64+d, head-major
            kT_sb = singles.tile([P, JT, S], BF16)
            o2T_sb = singles.tile([P, JT, S], BF16)  # normalized attn out
            v_sb = singles.tile([P, ET, HL, D], BF16)  # [sk_p, sk_tile, head, d]

            # ---- boot: jt0 q/k chains, kt-OUTER so they ride the x stream ----
            psl_q = ps_l.tile([P, 1024], F32, tag="psl", name="boot_q")
            psl_k = ps_l.tile([P, 1024], F32, tag="psl", name="boot_k")
            for kt in range(ET):
                for w_sb, pslx in ((wq_sb, psl_q), (wk_sb, psl_k)):
                    for nb in range(2):
                        nc.tensor.matmul(
                            pslx[:, nb * 512:(nb + 1) * 512],
                            lhsT=w_sb[:, kt, 0:P],
                            rhs=xT_sb[:, kt, nb * 512:(nb + 1) * 512],
                            start=(kt == 0), stop=(kt == ET - 1),
                            skip_group_check=True,
                        )
            for pslx, b_sb, dst in ((psl_q, bq_sb, qT_sb), (psl_k, bk_sb, kT_sb)):
                for nb in range(2):
                    nc.vector.tensor_scalar_add(
                        dst[:, 0, nb * 512:(nb + 1) * 512],
                        pslx[:, nb * 512:(nb + 1) * 512],
                        b_sb[:, 0:1],
                    )

            # ---- remaining QKV projections (kt-inner; x is resident) ----
            def emit_qk_chains(jt):
                for w_sb, b_sb, dst in ((wq_sb, bq_sb, qT_sb), (wk_sb, bk_sb, kT_sb)):
                    pss = [ps_mm.tile([P, 512], F32, tag="mm", name=f"mm_{jt}_{nb}")
                           for nb in range(2)]
                    for kt in range(ET):
                        for nb in range(2):
                            nc.tensor.matmul(
                                pss[nb][:],
                                lhsT=w_sb[:, kt, jt * P:(jt + 1) * P],
                                rhs=xT_sb[:, kt, nb * 512:(nb + 1) * 512],
                                start=(kt == 0), stop=(kt == ET - 1),
                            )
                    for nb in range(2):
                        nc.vector.tensor_scalar_add(
                            dst[:, jt, nb * 512:(nb + 1) * 512], pss[nb][:],
                            b_sb[:, jt:jt + 1],
                        )

            def emit_v_chains():
                for st in range(ET):
                    ps = ps_mm.tile([P, 512], F32, tag="mm", name=f"mmv_{st}")
                    for kt in range(ET):
                        nc.tensor.matmul(
                            ps[:],
                            lhsT=xT_sb[:, kt, st * P:(st + 1) * P],
                            rhs=wv_sb[:, kt, :],
                            start=(kt == 0), stop=(kt == ET - 1),
                        )
                    nc.vector.tensor_copy(
                        out=v_sb[:, st],
                        in_=ps[:].rearrange("p (h d) -> p h d", h=HL),
                    )

            # ---- attention in head pairs (rows 0-63 / 64-127 of the array) ----
            def pair_views(p):
                views = []
                for hh in (2 * p, 2 * p + 1):
                    po = (hh % 2) * 64
                    views.append((
                        qT_sb[po:po + 64, p, :],
                        kT_sb[po:po + 64, p, :],
                        pt_pool.tile([P, ET, S], BF16, tag="pt", name=f"pt_{hh}"),
                    ))
                return views

            def mask_diag(pT, t0, nt):
                diag = bass.AP(tensor=pT.tensor, offset=pT.offset + t0 * (S + P),
                               ap=[list(pT.ap[0]), [S + P, nt], [1, P]])
                nc.vector.tensor_mul(
                    out=diag, in0=diag,
                    in1=mask_sb[:, None, :].to_broadcast([P, nt, P]),
                )

            def emit_qk_pair(p):
                views = pair_views(p)
                for t in range(ET):
                    lo = t * P
                    psls = [ps_l.tile([P, 1024], F32, tag="psl",
                                      name=f"psl_{p}_{t}_{j}") for j in range(2)]
                    for cb in range(2):
                        c0, c1 = cb * 512, (cb + 1) * 512
                        s0 = max(lo, c0)
                        if s0 >= c1:
                            continue
                        for (qh, kh, _), psl in zip(views, psls):
                            nc.tensor.matmul(
                                psl[:, s0:c1],
                                lhsT=kh[:, lo:lo + P],
                                rhs=qh[:, s0:c1],
                                start=True, stop=True,
                            )
                    for (_, _, pT), psl in zip(views, psls):
                        nc.scalar.activation(
                            out=pT[:, t, lo:S], in_=psl[:, lo:S],
                            func=EXP, scale=0.125,
                        )
                for _, _, pT in views:
                    mask_diag(pT, 0, ET)
                return [v[2] for v in views]

            # last pair: QK+exp in sq-half chunks so PV/proj of half 0 can
            # start while half 1 is still exp-ing
            def emit_qk_pair_last_half(p, views, sqb):
                c0, c1 = sqb * 512, (sqb + 1) * 512
                for t in range(ET):
                    lo = t * P
                    s0 = max(lo, c0)
                    if s0 >= c1:
                        continue
                    for hh, (qh, kh, pT) in enumerate(views):
                        pslc = ps_l.tile([P, 512], F32, tag="psl",
                                         name=f"pslc_{sqb}_{t}_{hh}")
                        nc.tensor.matmul(
                            pslc[:, s0 - c0:512],
                            lhsT=kh[:, lo:lo + P],
                            rhs=qh[:, s0:c1],
                            start=True, stop=True,
                        )
                        nc.scalar.activation(
                            out=pT[:, t, s0:c1], in_=pslc[:, s0 - c0:512],
                            func=EXP, scale=0.125,
                        )
                for _, _, pT in views:
                    mask_diag(pT, 4 * sqb, 4)

            def emit_pv(p, pTs, sqb):
                c0, c1 = sqb * 512, (sqb + 1) * 512
                ts = [t for t in range(ET) if t * P < c1]
                pso = ps_o.tile([P, 512], F32, tag="pso", name=f"pso_{p}_{sqb}")
                for i, t in enumerate(ts):
                    s0 = max(t * P, c0)
                    off = s0 - c0
                    for hh in range(2):
                        nc.tensor.matmul(
                            pso[hh * 64:(hh + 1) * 64, off:512],
                            lhsT=v_sb[:, t, 2 * p + hh, :],
                            rhs=pTs[hh][:, t, s0:c1],
                            start=(i == 0), stop=(i == len(ts) - 1),
                            skip_group_check=True,
                        )
                den_ps = ps_o.tile([P, 512], F32, tag="pso", name=f"den_{p}_{sqb}")
                for i, t in enumerate(ts):
                    s0 = max(t * P, c0)
                    off = s0 - c0
                    for hh in range(2):
                        nc.tensor.matmul(
                            den_ps[hh * 64:hh * 64 + 1, off:512],
                            lhsT=ones_sb[:, :],
                            rhs=pTs[hh][:, t, s0:c1],
                            start=(i == 0), stop=(i == len(ts) - 1),
                            skip_group_check=True,
                        )
                den_a = den_pool.tile([1, 512], BF16, tag="dena", name=f"dena_{p}_{sqb}")
                den_b = den_pool.tile([1, 512], BF16, tag="denb", name=f"denb_{p}_{sqb}")
                nc.vector.tensor_copy(out=den_a[:, :], in_=den_ps[0:1, :])
                nc.vector.tensor_copy(out=den_b[:, :], in_=den_ps[64:65, :])
                bc_ps = ps_mm.tile([P, 512], F32, tag="mm", name=f"bcp_{p}_{sqb}")
                nc.tensor.matmul(bc_ps[0:64, :], lhsT=ones_row[:, :], rhs=den_a[:, :],
                                 start=True, stop=True, skip_group_check=True)
                nc.tensor.matmul(bc_ps[64:P, :], lhsT=ones_row[:, :], rhs=den_b[:, :],
                                 start=True, stop=True, skip_group_check=True)
                bc_f = bc_pool.tile([P, 512], F32, tag="bcf", name=f"bcf_{p}_{sqb}")
                nc.vector.reciprocal_approx_fast(out=bc_f[:, :], in_=bc_ps[:, :])
                nc.vector.tensor_mul(
                    out=o2T_sb[:, p, c0:c1], in0=pso[:, :], in1=bc_f[:, :],
                )

            def emit_proj(st):
                obt = out_pool.tile([P, E], BF16, tag="ob", name=f"ob_{st}")
                for eb in range(2):
                    psf = ps_mm.tile([P, 512], F32, tag="mm", name=f"mmp_{st}_{eb}")
                    for kt in range(JT):
                        nc.tensor.matmul(
                            psf[:],
                            lhsT=o2T_sb[:, kt, st * P:(st + 1) * P],
                            rhs=wp_sb[:, kt, eb * 512:(eb + 1) * 512],
                            start=(kt == 0), stop=(kt == JT - 1),
                        )
                    nc.vector.tensor_copy(
                        out=obt[:, eb * 512:(eb + 1) * 512], in_=psf[:])
                nc.sync.dma_start(out=out[st * P:(st + 1) * P, :], in_=obt[:])

            # ---- master pipeline ----
            pair0 = emit_qk_pair(0)
            emit_qk_chains(1)
            emit_v_chains()
            pair1 = emit_qk_pair(1)
            emit_pv(0, pair0, 0)
            emit_pv(0, pair0, 1)
            emit_qk_chains(2)
            pair2 = emit_qk_pair(2)
            emit_pv(1, pair1, 0)
            emit_pv(1, pair1, 1)
            emit_qk_chains(3)
            emit_pv(2, pair2, 0)
            pair3 = pair_views(3)
            pair3_pts = [v[2] for v in pair3]
            emit_qk_pair_last_half(3, pair3, 0)
            emit_pv(2, pair2, 1)
            emit_pv(3, pair3_pts, 0)
            for st in range(4):
                emit_proj(st)
            emit_qk_pair_last_half(3, pair3, 1)
            emit_pv(3, pair3_pts, 1)
            for st in range(4, ET):
                emit_proj(st)

    nc.compile()
    return nc


def make_in_maps(x, W_attn, b_attn, W_proj, b_proj):
    bf16 = ml_dtypes.bfloat16
    x = np.asarray(x, dtype=np.float32)
    W_attn = np.asarray(W_attn, dtype=np.float32)
    b_attn = np.asarray(b_attn, dtype=np.float32)
    W_proj = np.asarray(W_proj, dtype=np.float32)
    in_maps = []
    for i in range(NCORES):
        b, g = i // 2, i % 2
        j0 = g * JL
        in_maps.append({
            "xT": np.ascontiguousarray(x[b].T).astype(bf16),
            "wq": W_attn[:, j0:j0 + JL].astype(bf16),
            "wk": W_attn[:, E + j0:E + j0 + JL].astype(bf16),
            "wv": W_attn[:, 2 * E + j0:2 * E + j0 + JL].astype(bf16),
            "wp": W_proj[j0:j0 + JL, :].astype(bf16),
            "bq": np.ascontiguousarray(
                b_attn[j0:j0 + JL].astype(np.float32).reshape(JT, P).T),
            "bk": np.ascontiguousarray(
                b_attn[E + j0:E + j0 + JL].astype(np.float32).reshape(JT, P).T),
        })
    return in_maps


def kernel(x, W_attn, b_attn, W_proj, b_proj):
    global _NC_CACHE
    x = np.asarray(x, dtype=np.float32)
    W_attn = np.asarray(W_attn, dtype=np.float32)
    b_attn = np.asarray(b_attn, dtype=np.float32)
    W_proj = np.asarray(W_proj, dtype=np.float32)
    b_proj = np.asarray(b_proj, dtype=np.float32)

    if _NC_CACHE is None:
        _NC_CACHE = build_nc()
    nc = _NC_CACHE

    in_maps = make_in_maps(x, W_attn, b_attn, W_proj, b_proj)
    res = run_bass_kernel_spmd(nc, in_maps, core_ids=list(range(NCORES)))

    # host unshard: sum the two head-group partials + exact bias corrections
    bias_row = b_proj.copy()
    for g in range(2):
        j0 = g * JL
        bv = b_attn[2 * E + j0:2 * E + j0 + JL].astype(np.float32)
        bias_row += bv @ W_proj[j0:j0 + JL, :].astype(np.float32)

    full = np.empty((B, S, E), np.float32)
    for b in range(B):
        full[b] = (res.results[2 * b]["out"].astype(np.float32)
                   + res.results[2 * b + 1]["out"].astype(np.float32)
                   + bias_row[None, :])
    return full


# revision 8
# speedup vs baseline: 1.1869x; 1.1869x over previous
"""Causal multi-head attention block (B=4, S=1024, E=1024, H=16, D=64) on 8 TRN2 cores.

Sharding: data-parallel over batch (4) x tensor-parallel over heads (2 groups of 8).
Core i handles batch i//2, head-group i%2. Each core computes its partial output
projection (row-parallel W_proj); the host sums the two TP partials per batch and
applies the (exact) bias corrections.

Device-side math per core (bf16 compute, f32 accumulate):
  qT = (Wq_g)^T x^T + bq_g          [512, 1024]  (head-major rows h*64+d)
  kT = (Wk_g)^T x^T + bk_g          [512, 1024]
  v  = x Wv_g                       [1024, 512]
  Attention runs in head PAIRS. QK^T is computed per (sk-tile, sq-half) chunk
  with the two heads in the two row-halves of the PE array; both heads' scores
  exp() in ONE strided ScalarE activation into a pair-shared PT tile
  [sk, t, head, sq]. sq-half 0 chunks come first so PV of half 0 overlaps the
  pair's remaining exps. PV + softmax-denominator matmuls are column-tiled
  (concurrent) pairs; a K=1 ones-matmul pair broadcasts the denominators
  across partitions; one reciprocal + one multiply normalizes.
  out_partial = o2T^T Wp_g          [1024, 1024]  (bf16 to DRAM)
Host: out[b] = out_partial[2b] + out_partial[2b+1] + (bv_0 Wp_0 + bv_1 Wp_1 + b_proj)
(the v-bias term is exact because softmax rows sum to 1).
"""

import numpy as np
import ml_dtypes

import concourse.bass as bass
import concourse.tile as tile
from concourse import bacc, mybir
from concourse.bass_utils import run_bass_kernel_spmd
from concourse.masks import make_upper_triangular

BF16 = mybir.dt.bfloat16
F32 = mybir.dt.float32

B, S, E = 4, 1024, 1024
H_TOT, D = 16, 64
NCORES = 8
HL = 8            # heads per core
JL = HL * D       # 512 local qkv dim
P = 128
ET = E // P       # 8 k-tiles over embed dim
JT = JL // P      # 4 partition-tiles over local qkv dim
EXP = mybir.ActivationFunctionType.Exp

_NC_CACHE = None


def build_nc():
    nc = bacc.Bacc()

    xT = nc.declare_dram_parameter("xT", [E, S], BF16, isOutput=False)
    wq = nc.declare_dram_parameter("wq", [E, JL], BF16, isOutput=False)
    wk = nc.declare_dram_parameter("wk", [E, JL], BF16, isOutput=False)
    wv = nc.declare_dram_parameter("wv", [E, JL], BF16, isOutput=False)
    wp = nc.declare_dram_parameter("wp", [JL, E], BF16, isOutput=False)
    bq = nc.declare_dram_parameter("bq", [P, JT], F32, isOutput=False)
    bk = nc.declare_dram_parameter("bk", [P, JT], F32, isOutput=False)
    out = nc.declare_dram_parameter("out", [S, E], BF16, isOutput=True)

    with tile.TileContext(nc) as tc:
        with (
            tc.tile_pool(name="singles", bufs=1) as singles,
            tc.tile_pool(name="pt", bufs=3) as pt_pool,
            tc.tile_pool(name="den", bufs=2) as den_pool,
            tc.tile_pool(name="bc", bufs=2) as bc_pool,
            tc.tile_pool(name="outst", bufs=2) as out_pool,
            tc.tile_pool(name="ps_l", bufs=2, space="PSUM") as ps_l,
            tc.tile_pool(name="ps_mm", bufs=2, space="PSUM") as ps_mm,
            tc.tile_pool(name="ps_o", bufs=2, space="PSUM") as ps_o,
        ):
            # ---- static inputs -> SBUF ----
            xT_sb = singles.tile([P, ET, S], BF16)
            wq_sb = singles.tile([P, ET, JL], BF16)
            wk_sb = singles.tile([P, ET, JL], BF16)
            wv_sb = singles.tile([P, ET, JL], BF16)
            wp_sb = singles.tile([P, JT, E], BF16)
            bq_sb = singles.tile([P, JT], F32)
            bk_sb = singles.tile([P, JT], F32)
            xT_r = xT[:, :].rearrange("(o p) s -> p o s", p=P)
            wq_r = wq[:, :].rearrange("(o p) j -> p o j", p=P)
            wk_r = wk[:, :].rearrange("(o p) j -> p o j", p=P)
            wv_r = wv[:, :].rearrange("(o p) j -> p o j", p=P)
            wp_r = wp[:, :].rearrange("(o p) e -> p o e", p=P)

            # x streams per-ktile on the sync queue; the boot q/k chains
            # (jt0, kt-outer) consume tiles as they land
            for kt in range(ET):
                nc.sync.dma_start(out=xT_sb[:, kt], in_=xT_r[:, kt])
            # scalar (HWDGE) queue: jt0 slices of wq/wk first so the boot
            # chains start as soon as x tile 0 lands
            nc.scalar.dma_start(out=wq_sb[:, :, 0:P], in_=wq_r[:, :, 0:P])
            nc.scalar.dma_start(out=wk_sb[:, :, 0:P], in_=wk_r[:, :, 0:P])
            nc.scalar.dma_start(out=wq_sb[:, :, P:JL], in_=wq_r[:, :, P:JL])
            nc.scalar.dma_start(out=wk_sb[:, :, P:JL], in_=wk_r[:, :, P:JL])
            # gpsimd (SWDGE) queue: biases, then wv (needed ~15us), then wp
            nc.gpsimd.dma_start(out=bq_sb[:], in_=bq[:, :])
            nc.gpsimd.dma_start(out=bk_sb[:], in_=bk[:, :])
            for c in range(0, ET, 4):
                nc.gpsimd.dma_start(out=wv_sb[:, c:c + 4], in_=wv_r[:, c:c + 4])
            for c in range(0, JT, 2):
                nc.gpsimd.dma_start(out=wp_sb[:, c:c + 2], in_=wp_r[:, c:c + 2])

            # pre-trigger the exp ACT table load (~2.7us) during the DMA wait
            warm_in = singles.tile([1, 1], F32)
            warm_out = singles.tile([1, 1], F32)
            nc.vector.memset(warm_in[:, :], 0.0)
            nc.scalar.activation(out=warm_out[:, :], in_=warm_in[:, :], func=EXP)

            # causal keep-mask for diagonal PT blocks: 1 where sq >= sk else 0
            mask_sb = singles.tile([P, P], BF16)
            make_upper_triangular(nc, mask_sb[:], val=1.0, diag=True)

            ones_sb = singles.tile([P, 1], BF16)   # denominator matmuls
            nc.vector.memset(ones_sb[:, :], 1.0)
            ones_row = singles.tile([1, 64], BF16)  # K=1 broadcast matmuls
            nc.vector.memset(ones_row[:, :], 1.0)

            qT_sb = singles.tile([P, JT, S], BF16)   # row j = h*64+d, head-major
            kT_sb = singles.tile([P, JT, S], BF16)
            o2T_sb = singles.tile([P, JT, S], BF16)  # normalized attn out
            v_sb = singles.tile([P, ET, HL, D], BF16)  # [sk_p, sk_tile, head, d]

            # ---- boot: jt0 q/k chains, kt-OUTER so they ride the x stream ----
            psl_q = ps_l.tile([P, 2, 512], F32, tag="psl", name="boot_q")
            psl_k = ps_l.tile([P, 2, 512], F32, tag="psl", name="boot_k")
            for kt in range(ET):
                for w_sb, pslx in ((wq_sb, psl_q), (wk_sb, psl_k)):
                    for nb in range(2):
                        nc.tensor.matmul(
                            pslx[:, nb, :],
                            lhsT=w_sb[:, kt, 0:P],
                            rhs=xT_sb[:, kt, nb * 512:(nb + 1) * 512],
                            start=(kt == 0), stop=(kt == ET - 1),
                            skip_group_check=True,
                        )
            for pslx, b_sb, dst in ((psl_q, bq_sb, qT_sb), (psl_k, bk_sb, kT_sb)):
                for nb in range(2):
                    nc.vector.tensor_scalar_add(
                        dst[:, 0, nb * 512:(nb + 1) * 512],
                        pslx[:, nb, :],
                        b_sb[:, 0:1],
                    )

            # ---- filler units (interleaved between attention steps) ----
            def emit_chain(jt, which):
                w_sb, b_sb, dst = ((wq_sb, bq_sb, qT_sb) if which == "q"
                                   else (wk_sb, bk_sb, kT_sb))
                pss = [ps_mm.tile([P, 512], F32, tag="mm",
                                  name=f"mm_{which}{jt}_{nb}") for nb in range(2)]
                for kt in range(ET):
                    for nb in range(2):
                        nc.tensor.matmul(
                            pss[nb][:],
                            lhsT=w_sb[:, kt, jt * P:(jt + 1) * P],
                            rhs=xT_sb[:, kt, nb * 512:(nb + 1) * 512],
                            start=(kt == 0), stop=(kt == ET - 1),
                        )
                for nb in range(2):
                    nc.vector.tensor_scalar_add(
                        dst[:, jt, nb * 512:(nb + 1) * 512], pss[nb][:],
                        b_sb[:, jt:jt + 1],
                    )

            def emit_v(st):
                ps = ps_mm.tile([P, 512], F32, tag="mm", name=f"mmv_{st}")
                for kt in range(ET):
                    nc.tensor.matmul(
                        ps[:],
                        lhsT=xT_sb[:, kt, st * P:(st + 1) * P],
                        rhs=wv_sb[:, kt, :],
                        start=(kt == 0), stop=(kt == ET - 1),
                    )
                nc.vector.tensor_copy(
                    out=v_sb[:, st],
                    in_=ps[:].rearrange("p (h d) -> p h d", h=HL),
                )

            # ---- attention in head pairs ----
            # pair p: heads (2p, 2p+1); PT stored pair-shared [sk, t, head, sq]
            def new_ptp(p):
                return pt_pool.tile([P, ET, 2, S], BF16, tag="pt", name=f"ptp_{p}")

            def emit_qk_half(p, ptp, half):
                # half 0: chunks (t=0..3, sq 0-511); half 1: (t=0..3, sq
                # 512-1023) + (t=4..7, sq lo-1023). Both heads' chunk goes
                # through one strided exp.
                chunks = ([(t, 0) for t in range(4)] if half == 0
                          else [(t, 1) for t in range(ET)])
                for t, cbi in chunks:
                    lo = t * P
                    s0 = max(lo, 512 * cbi)
                    c1 = 512 * (cbi + 1)
                    off = s0 - 512 * cbi
                    psl = ps_l.tile([P, 2, 512], F32, tag="psl",
                                    name=f"psl_{p}_{t}_{cbi}")
                    for hh in range(2):
                        po = hh * 64
                        nc.tensor.matmul(
                            psl[:, hh, off:512],
                            lhsT=kT_sb[po:po + 64, p, lo:lo + P],
                            rhs=qT_sb[po:po + 64, p, s0:c1],
                            start=True, stop=True,
                        )
                    nc.scalar.activation(
                        out=ptp[:, t, :, s0:c1], in_=psl[:, :, off:512],
                        func=EXP, scale=0.125,
                    )
                # mask the diagonal blocks of this half's t range, both heads
                t0 = 0 if half == 0 else 4
                for hh in range(2):
                    diag = bass.AP(
                        tensor=ptp.tensor,
                        offset=ptp.offset + hh * S + t0 * (2 * S + P),
                        ap=[list(ptp.ap[0]), [2 * S + P, 4], [1, P]])
                    nc.vector.tensor_mul(
                        out=diag, in0=diag,
                        in1=mask_sb[:, None, :].to_broadcast([P, 4, P]),
                    )

            def emit_pv(p, ptp, sqb):
                c0, c1 = sqb * 512, (sqb + 1) * 512
                ts = [t for t in range(ET) if t * P < c1]
                pso = ps_o.tile([P, 512], F32, tag="pso", name=f"pso_{p}_{sqb}")
                for i, t in enumerate(ts):
                    s0 = max(t * P, c0)
                    off = s0 - c0
                    for hh in range(2):
                        nc.tensor.matmul(
                            pso[hh * 64:(hh + 1) * 64, off:512],
                            lhsT=v_sb[:, t, 2 * p + hh, :],
                            rhs=ptp[:, t, hh, s0:c1],
                            start=(i == 0), stop=(i == len(ts) - 1),
                            skip_group_check=True,
                        )
                den_ps = ps_o.tile([P, 512], F32, tag="pso", name=f"den_{p}_{sqb}")
                for i, t in enumerate(ts):
                    s0 = max(t * P, c0)
                    off = s0 - c0
                    for hh in range(2):
                        nc.tensor.matmul(
                            den_ps[hh * 64:hh * 64 + 1, off:512],
                            lhsT=ones_sb[:, :],
                            rhs=ptp[:, t, hh, s0:c1],
                            start=(i == 0), stop=(i == len(ts) - 1),
                            skip_group_check=True,
                        )
                den_a = den_pool.tile([1, 512], BF16, tag="dena",
                                      name=f"dena_{p}_{sqb}")
                den_b = den_pool.tile([1, 512], BF16, tag="denb",
                                      name=f"denb_{p}_{sqb}")
                nc.vector.tensor_copy(out=den_a[:, :], in_=den_ps[0:1, :])
                nc.vector.tensor_copy(out=den_b[:, :], in_=den_ps[64:65, :])
                bc_ps = ps_mm.tile([P, 512], F32, tag="mm", name=f"bcp_{p}_{sqb}")
                nc.tensor.matmul(bc_ps[0:64, :], lhsT=ones_row[:, :],
                                 rhs=den_a[:, :],
                                 start=True, stop=True, skip_group_check=True)
                nc.tensor.matmul(bc_ps[64:P, :], lhsT=ones_row[:, :],
                                 rhs=den_b[:, :],
                                 start=True, stop=True, skip_group_check=True)
                bc_f = bc_pool.tile([P, 512], F32, tag="bcf", name=f"bcf_{p}_{sqb}")
                nc.vector.reciprocal_approx_fast(out=bc_f[:, :], in_=bc_ps[:, :])
                nc.vector.tensor_mul(
                    out=o2T_sb[:, p, c0:c1], in0=pso[:, :], in1=bc_f[:, :],
                )

            def emit_proj(st):
                obt = out_pool.tile([P, E], BF16, tag="ob", name=f"ob_{st}")
                for eb in range(2):
                    psf = ps_mm.tile([P, 512], F32, tag="mm", name=f"mmp_{st}_{eb}")
                    for kt in range(JT):
                        nc.tensor.matmul(
                            psf[:],
                            lhsT=o2T_sb[:, kt, st * P:(st + 1) * P],
                            rhs=wp_sb[:, kt, eb * 512:(eb + 1) * 512],
                            start=(kt == 0), stop=(kt == JT - 1),
                        )
                    nc.vector.tensor_copy(
                        out=obt[:, eb * 512:(eb + 1) * 512], in_=psf[:])
                nc.sync.dma_start(out=out[st * P:(st + 1) * P, :], in_=obt[:])

            # ---- master pipeline: QK/exp chunks with filler work interleaved
            # so the PE never drains while ScalarE grinds through the exps ----
            ptps = [new_ptp(0), new_ptp(1)]

            emit_qk_half(0, ptps[0], 0)
            emit_chain(1, "q")
            emit_qk_half(0, ptps[0], 1)
            emit_chain(1, "k")
            emit_v(0)
            emit_v(1)
            emit_v(2)
            emit_v(3)
            emit_pv(0, ptps[0], 0)
            emit_qk_half(1, ptps[1], 0)
            emit_v(4)
            emit_v(5)
            emit_v(6)
            emit_v(7)
            emit_chain(2, "q")
            emit_pv(0, ptps[0], 1)
            emit_qk_half(1, ptps[1], 1)
            emit_chain(2, "k")
            emit_pv(1, ptps[1], 0)
            ptps.append(new_ptp(2))
            emit_qk_half(2, ptps[2], 0)
            emit_chain(3, "q")
            emit_pv(1, ptps[1], 1)
            emit_qk_half(2, ptps[2], 1)
            emit_chain(3, "k")
            emit_pv(2, ptps[2], 0)
            ptps.append(new_ptp(3))
            emit_qk_half(3, ptps[3], 0)
            emit_pv(2, ptps[2], 1)
            emit_pv(3, ptps[3], 0)
            emit_qk_half(3, ptps[3], 1)
            for st in range(4):
                emit_proj(st)
            emit_pv(3, ptps[3], 1)
            for st in range(4, ET):
                emit_proj(st)

    nc.compile()
    return nc


def make_in_maps(x, W_attn, b_attn, W_proj, b_proj):
    bf16 = ml_dtypes.bfloat16
    x = np.asarray(x, dtype=np.float32)
    W_attn = np.asarray(W_attn, dtype=np.float32)
    b_attn = np.asarray(b_attn, dtype=np.float32)
    W_proj = np.asarray(W_proj, dtype=np.float32)
    in_maps = []
    for i in range(NCORES):
        b, g = i // 2, i % 2
        j0 = g * JL
        in_maps.append({
            "xT": np.ascontiguousarray(x[b].T).astype(bf16),
            "wq": W_attn[:, j0:j0 + JL].astype(bf16),
            "wk": W_attn[:, E + j0:E + j0 + JL].astype(bf16),
            "wv": W_attn[:, 2 * E + j0:2 * E + j0 + JL].astype(bf16),
            "wp": W_proj[j0:j0 + JL, :].astype(bf16),
            "bq": np.ascontiguousarray(
                b_attn[j0:j0 + JL].astype(np.float32).reshape(JT, P).T),
            "bk": np.ascontiguousarray(
                b_attn[E + j0:E + j0 + JL].astype(np.float32).reshape(JT, P).T),
        })
    return in_maps


def kernel(x, W_attn, b_attn, W_proj, b_proj):
    global _NC_CACHE
    x = np.asarray(x, dtype=np.float32)
    W_attn = np.asarray(W_attn, dtype=np.float32)
    b_attn = np.asarray(b_attn, dtype=np.float32)
    W_proj = np.asarray(W_proj, dtype=np.float32)
    b_proj = np.asarray(b_proj, dtype=np.float32)

    if _NC_CACHE is None:
        _NC_CACHE = build_nc()
    nc = _NC_CACHE

    in_maps = make_in_maps(x, W_attn, b_attn, W_proj, b_proj)
    res = run_bass_kernel_spmd(nc, in_maps, core_ids=list(range(NCORES)))

    # host unshard: sum the two head-group partials + exact bias corrections
    bias_row = b_proj.copy()
    for g in range(2):
        j0 = g * JL
        bv = b_attn[2 * E + j0:2 * E + j0 + JL].astype(np.float32)
        bias_row += bv @ W_proj[j0:j0 + JL, :].astype(np.float32)

    full = np.empty((B, S, E), np.float32)
    for b in range(B):
        full[b] = (res.results[2 * b]["out"].astype(np.float32)
                   + res.results[2 * b + 1]["out"].astype(np.float32)
                   + bias_row[None, :])
    return full


# revision 13
# speedup vs baseline: 1.4266x; 1.2020x over previous
"""Causal multi-head attention block (B=4, S=1024, E=1024, H=16, D=64) on 8 TRN2 cores.

Sharding: data-parallel over batch (4) x tensor-parallel over heads (2 groups of 8).
Core i handles batch i//2, head-group i%2. Each core computes its partial output
projection (row-parallel W_proj); the host sums the two TP partials per batch and
applies the (exact) bias corrections.

Device-side math per core (bf16 compute, f32 accumulate):
  qT = (Wq_g)^T x^T + bq_g          [512, 1024]  (head-major rows h*64+d)
  kT = (Wk_g)^T x^T + bk_g          [512, 1024]
  v  = x Wv_g                       [1024, 512]
  Attention runs in head PAIRS. QK^T is computed per (sk-tile, sq-half) chunk
  with the two heads in the two row-halves of the PE array; both heads' scores
  exp() in ONE strided ScalarE activation into a pair-shared PT tile
  [sk, t, head, sq]. sq-half 0 chunks come first so PV of half 0 overlaps the
  pair's remaining exps. PV + softmax-denominator matmuls are column-tiled
  (concurrent) pairs; a K=1 ones-matmul pair broadcasts the denominators
  across partitions; one reciprocal + one multiply normalizes.
  out_partial = o2T^T Wp_g          [1024, 1024]  (bf16 to DRAM)
Host: out[b] = out_partial[2b] + out_partial[2b+1] + (bv_0 Wp_0 + bv_1 Wp_1 + b_proj)
(the v-bias term is exact because softmax rows sum to 1).
"""

import numpy as np
import ml_dtypes

import concourse.bass as bass
import concourse.tile as tile
from concourse import bacc, mybir
from concourse.bass_utils import run_bass_kernel_spmd
from concourse.masks import make_upper_triangular

BF16 = mybir.dt.bfloat16
F32 = mybir.dt.float32

B, S, E = 4, 1024, 1024
H_TOT, D = 16, 64
NCORES = 8
HL = 8            # heads per core
JL = HL * D       # 512 local qkv dim
P = 128
ET = E // P       # 8 k-tiles over embed dim
JT = JL // P      # 4 partition-tiles over local qkv dim
EXP = mybir.ActivationFunctionType.Exp

_NC_CACHE = None


def build_nc():
    nc = bacc.Bacc()

    xT = nc.declare_dram_parameter("xT", [E, S], BF16, isOutput=False)
    wq = nc.declare_dram_parameter("wq", [E, JL], BF16, isOutput=False)
    wk = nc.declare_dram_parameter("wk", [E, JL], BF16, isOutput=False)
    wv = nc.declare_dram_parameter("wv", [E, JL], BF16, isOutput=False)
    wp = nc.declare_dram_parameter("wp", [JL, E], BF16, isOutput=False)
    bq = nc.declare_dram_parameter("bq", [P, JT], F32, isOutput=False)
    bk = nc.declare_dram_parameter("bk", [P, JT], F32, isOutput=False)
    out = nc.declare_dram_parameter("out", [S, E], BF16, isOutput=True)

    with tile.TileContext(nc) as tc:
        with (
            tc.tile_pool(name="singles", bufs=1) as singles,
            tc.tile_pool(name="pt", bufs=3) as pt_pool,
            tc.tile_pool(name="bc", bufs=2) as bc_pool,
            tc.tile_pool(name="outst", bufs=2) as out_pool,
            tc.tile_pool(name="ps_l", bufs=2, space="PSUM") as ps_l,
            tc.tile_pool(name="ps_mm", bufs=2, space="PSUM") as ps_mm,
            tc.tile_pool(name="ps_o", bufs=2, space="PSUM") as ps_o,
        ):
            # ---- static inputs -> SBUF ----
            xT_sb = singles.tile([P, ET, S], BF16)
            wq_sb = singles.tile([P, ET, JL], BF16)
            wk_sb = singles.tile([P, ET, JL], BF16)
            wv_sb = singles.tile([P, ET, JL], BF16)
            wp_sb = singles.tile([P, JT, E], BF16)
            bq_sb = singles.tile([P, JT], F32)
            bk_sb = singles.tile([P, JT], F32)
            xT_r = xT[:, :].rearrange("(o p) s -> p o s", p=P)
            wq_r = wq[:, :].rearrange("(o p) j -> p o j", p=P)
            wk_r = wk[:, :].rearrange("(o p) j -> p o j", p=P)
            wv_r = wv[:, :].rearrange("(o p) j -> p o j", p=P)
            wp_r = wp[:, :].rearrange("(o p) e -> p o e", p=P)

            # x streams per-ktile on the sync queue; the boot q/k chains
            # (jt0, kt-outer) consume tiles as they land
            for kt in range(ET):
                nc.sync.dma_start(out=xT_sb[:, kt], in_=xT_r[:, kt])
            # scalar (HWDGE) queue: jt0 slices of wq/wk first so the boot
            # chains start as soon as x tile 0 lands
            nc.scalar.dma_start(out=wq_sb[:, :, 0:P], in_=wq_r[:, :, 0:P])
            nc.scalar.dma_start(out=wk_sb[:, :, 0:P], in_=wk_r[:, :, 0:P])
            nc.scalar.dma_start(out=wq_sb[:, :, P:JL], in_=wq_r[:, :, P:JL])
            nc.scalar.dma_start(out=wk_sb[:, :, P:JL], in_=wk_r[:, :, P:JL])
            # gpsimd (SWDGE) queue: biases, then wv (needed ~15us), then wp
            nc.gpsimd.dma_start(out=bq_sb[:], in_=bq[:, :])
            nc.gpsimd.dma_start(out=bk_sb[:], in_=bk[:, :])
            for c in range(0, ET, 4):
                nc.gpsimd.dma_start(out=wv_sb[:, c:c + 4], in_=wv_r[:, c:c + 4])
            for c in range(0, JT, 2):
                nc.gpsimd.dma_start(out=wp_sb[:, c:c + 2], in_=wp_r[:, c:c + 2])

            # pre-trigger the exp ACT table load (~2.7us) during the DMA wait
            warm_in = singles.tile([1, 1], F32)
            warm_out = singles.tile([1, 1], F32)
            nc.vector.memset(warm_in[:, :], 0.0)
            nc.scalar.activation(out=warm_out[:, :], in_=warm_in[:, :], func=EXP)

            # causal keep-mask for diagonal PT blocks: 1 where sq >= sk else 0
            mask_sb = singles.tile([P, P], BF16)
            make_upper_triangular(nc, mask_sb[:], val=1.0, diag=True)

            # all-ones stationary: the denominator matmul with this lhsT
            # yields the denominator already broadcast across 64 partitions
            ones64 = singles.tile([P, 64], BF16)
            nc.vector.memset(ones64[:, :], 1.0)

            qT_sb = singles.tile([P, JT, S], BF16)   # row j = h*64+d, head-major
            kT_sb = singles.tile([P, JT, S], BF16)
            o2T_sb = singles.tile([P, JT, S], BF16)  # normalized attn out
            v_sb = singles.tile([P, ET, HL, D], BF16)  # [sk_p, sk_tile, head, d]

            # ---- boot: jt0 q/k chains, kt-OUTER so they ride the x stream ----
            psl_q = ps_l.tile([P, 2, 512], F32, tag="psl", name="boot_q")
            psl_k = ps_l.tile([P, 2, 512], F32, tag="psl", name="boot_k")
            for kt in range(ET):
                for w_sb, pslx in ((wq_sb, psl_q), (wk_sb, psl_k)):
                    for nb in range(2):
                        nc.tensor.matmul(
                            pslx[:, nb, :],
                            lhsT=w_sb[:, kt, 0:P],
                            rhs=xT_sb[:, kt, nb * 512:(nb + 1) * 512],
                            start=(kt == 0), stop=(kt == ET - 1),
                            skip_group_check=True,
                        )
            for pslx, b_sb, dst in ((psl_q, bq_sb, qT_sb), (psl_k, bk_sb, kT_sb)):
                for nb in range(2):
                    nc.vector.tensor_scalar_add(
                        dst[:, 0, nb * 512:(nb + 1) * 512],
                        pslx[:, nb, :],
                        b_sb[:, 0:1],
                    )

            # ---- filler units: small PE work packets (~1us each) that get
            # woven between the exp-gated QK chunks so the PE queue never
            # head-of-line blocks on ScalarE ----
            def chain_units(jt, which):
                w_sb, b_sb, dst = ((wq_sb, bq_sb, qT_sb) if which == "q"
                                   else (wk_sb, bk_sb, kT_sb))
                state = {}

                def step(kt0, kt1, first, last):
                    if first:
                        state["pss"] = [
                            ps_mm.tile([P, 512], F32, tag="mm",
                                       name=f"mm_{which}{jt}_{nb}")
                            for nb in range(2)]
                    for kt in range(kt0, kt1):
                        for nb in range(2):
                            nc.tensor.matmul(
                                state["pss"][nb][:],
                                lhsT=w_sb[:, kt, jt * P:(jt + 1) * P],
                                rhs=xT_sb[:, kt, nb * 512:(nb + 1) * 512],
                                start=(kt == 0), stop=(kt == ET - 1),
                            )
                    if last:
                        for nb in range(2):
                            nc.vector.tensor_scalar_add(
                                dst[:, jt, nb * 512:(nb + 1) * 512],
                                state["pss"][nb][:],
                                b_sb[:, jt:jt + 1],
                            )

                return [lambda k0=k0: step(k0, k0 + 2, k0 == 0, k0 == ET - 2)
                        for k0 in range(0, ET, 2)]

            def v_units(st):
                state = {}

                def step(kt0, kt1, first, last):
                    if first:
                        state["ps"] = ps_mm.tile([P, 512], F32, tag="mm",
                                                 name=f"mmv_{st}")
                    for kt in range(kt0, kt1):
                        nc.tensor.matmul(
                            state["ps"][:],
                            lhsT=xT_sb[:, kt, st * P:(st + 1) * P],
                            rhs=wv_sb[:, kt, :],
                            start=(kt == 0), stop=(kt == ET - 1),
                        )
                    if last:
                        nc.vector.tensor_copy(
                            out=v_sb[:, st],
                            in_=state["ps"][:].rearrange("p (h d) -> p h d", h=HL),
                        )

                return [lambda k0=k0: step(k0, k0 + 4, k0 == 0, k0 == 4)
                        for k0 in range(0, ET, 4)]

            # ---- attention in head pairs ----
            # pair p: heads (2p, 2p+1); PT stored pair-shared [sk, t, head, sq]
            def new_ptp(p):
                return pt_pool.tile([P, ET, 2, S], BF16, tag="pt", name=f"ptp_{p}")

            def emit_qk_half(p, ptp, half, filler):
                # half 0: chunks (t=0..3, sq 0-511); half 1: (t=0..3, sq
                # 512-1023) + (t=4..7, sq lo-1023). Both heads' chunk goes
                # through one strided exp. One filler unit is woven in after
                # each chunk to cover the exp shadow.
                chunks = ([(t, 0) for t in range(4)] if half == 0
                          else [(t, 1) for t in range(ET)])
                for t, cbi in chunks:
                    lo = t * P
                    s0 = max(lo, 512 * cbi)
                    c1 = 512 * (cbi + 1)
                    off = s0 - 512 * cbi
                    psl = ps_l.tile([P, 2, 512], F32, tag="psl",
                                    name=f"psl_{p}_{t}_{cbi}")
                    for hh in range(2):
                        po = hh * 64
                        nc.tensor.matmul(
                            psl[:, hh, off:512],
                            lhsT=kT_sb[po:po + 64, p, lo:lo + P],
                            rhs=qT_sb[po:po + 64, p, s0:c1],
                            start=True, stop=True,
                        )
                    nc.scalar.activation(
                        out=ptp[:, t, :, s0:c1], in_=psl[:, :, off:512],
                        func=EXP, scale=0.125,
                    )
                    if filler:
                        filler.popleft()()
                # mask the diagonal blocks of this half's t range, both heads
                t0 = 0 if half == 0 else 4
                for hh in range(2):
                    diag = bass.AP(
                        tensor=ptp.tensor,
                        offset=ptp.offset + hh * S + t0 * (2 * S + P),
                        ap=[list(ptp.ap[0]), [2 * S + P, 4], [1, P]])
                    nc.vector.tensor_mul(
                        out=diag, in0=diag,
                        in1=mask_sb[:, None, :].to_broadcast([P, 4, P]),
                    )

            def emit_pv(p, ptp, sqb):
                c0, c1 = sqb * 512, (sqb + 1) * 512
                ts = [t for t in range(ET) if t * P < c1]
                pso = ps_o.tile([P, 512], F32, tag="pso", name=f"pso_{p}_{sqb}")
                for i, t in enumerate(ts):
                    s0 = max(t * P, c0)
                    off = s0 - c0
                    for hh in range(2):
                        nc.tensor.matmul(
                            pso[hh * 64:(hh + 1) * 64, off:512],
                            lhsT=v_sb[:, t, 2 * p + hh, :],
                            rhs=ptp[:, t, hh, s0:c1],
                            start=(i == 0), stop=(i == len(ts) - 1),
                            skip_group_check=True,
                        )
                den_bc = ps_o.tile([P, 512], F32, tag="pso", name=f"den_{p}_{sqb}")
                for i, t in enumerate(ts):
                    s0 = max(t * P, c0)
                    off = s0 - c0
                    for hh in range(2):
                        nc.tensor.matmul(
                            den_bc[hh * 64:(hh + 1) * 64, off:512],
                            lhsT=ones64[:, :],
                            rhs=ptp[:, t, hh, s0:c1],
                            start=(i == 0), stop=(i == len(ts) - 1),
                            skip_group_check=True,
                        )
                bc_f = bc_pool.tile([P, 512], F32, tag="bcf", name=f"bcf_{p}_{sqb}")
                nc.vector.reciprocal_approx_fast(out=bc_f[:, :], in_=den_bc[:, :])
                nc.vector.tensor_mul(
                    out=o2T_sb[:, p, c0:c1], in0=pso[:, :], in1=bc_f[:, :],
                )

            def proj_units(st):
                state = {}

                def step(eb):
                    if eb == 0:
                        state["obt"] = out_pool.tile([P, E], BF16, tag="ob",
                                                     name=f"ob_{st}")
                    psf = ps_mm.tile([P, 512], F32, tag="mm", name=f"mmp_{st}_{eb}")
                    for kt in range(JT):
                        nc.tensor.matmul(
                            psf[:],
                            lhsT=o2T_sb[:, kt, st * P:(st + 1) * P],
                            rhs=wp_sb[:, kt, eb * 512:(eb + 1) * 512],
                            start=(kt == 0), stop=(kt == JT - 1),
                        )
                    if (st + eb) % 2 == 0:
                        nc.vector.tensor_copy(
                            out=state["obt"][:, eb * 512:(eb + 1) * 512],
                            in_=psf[:])
                    else:
                        nc.scalar.copy(
                            out=state["obt"][:, eb * 512:(eb + 1) * 512],
                            in_=psf[:])
                    if eb == 1:
                        nc.sync.dma_start(out=out[st * P:(st + 1) * P, :],
                                          in_=state["obt"][:])

                return [lambda e=e: step(e) for e in range(2)]

            # ---- master pipeline: per pair, emit QK/exp chunks (sq-half 0
            # first) with one filler unit after each chunk; PV/den of a half
            # as soon as its masks are in; projections interleave into the
            # last pair's half-1 chunks ----
            from collections import deque

            ptps = [new_ptp(0), new_ptp(1)]

            fill = deque()
            fill += v_units(0) + v_units(1) + v_units(2) + v_units(3)
            fill += chain_units(1, "q")
            emit_qk_half(0, ptps[0], 0, fill)        # 4 chunks: v0 v0 v1 v1
            emit_qk_half(0, ptps[0], 1, fill)        # 8: v2 v2 v3 v3 c1q x4
            fill += chain_units(1, "k")
            while fill:
                fill.popleft()()                     # c1k before pair 1
            emit_pv(0, ptps[0], 0)
            fill += v_units(4) + v_units(5) + v_units(6) + v_units(7)
            fill += chain_units(2, "q") + chain_units(2, "k")
            emit_qk_half(1, ptps[1], 0, fill)        # v4 v4 v5 v5
            emit_pv(1, ptps[1], 0)
            emit_qk_half(1, ptps[1], 1, fill)        # v6 v6 v7 v7 c2q x4
            emit_pv(0, ptps[0], 1)
            while fill:
                fill.popleft()()                     # c2k before pair 2
            emit_pv(1, ptps[1], 1)
            ptps.append(new_ptp(2))
            fill += chain_units(3, "q") + chain_units(3, "k")
            emit_qk_half(2, ptps[2], 0, fill)        # c3q x4
            emit_pv(2, ptps[2], 0)
            emit_qk_half(2, ptps[2], 1, fill)        # c3k x4 + 4 empty
            ptps.append(new_ptp(3))
            fill += [lambda: emit_pv(2, ptps[2], 1)]
            emit_qk_half(3, ptps[3], 0, fill)
            while fill:
                fill.popleft()()
            emit_pv(3, ptps[3], 0)
            fill += proj_units(0) + proj_units(1) + proj_units(2) + proj_units(3)
            emit_qk_half(3, ptps[3], 1, fill)        # proj 0-3 interleave
            while fill:
                fill.popleft()()
            emit_pv(3, ptps[3], 1)
            for st in range(4, ET):
                for u in proj_units(st):
                    u()

    nc.compile()
    return nc


def make_in_maps(x, W_attn, b_attn, W_proj, b_proj):
    bf16 = ml_dtypes.bfloat16
    x = np.asarray(x, dtype=np.float32)
    W_attn = np.asarray(W_attn, dtype=np.float32)
    b_attn = np.asarray(b_attn, dtype=np.float32)
    W_proj = np.asarray(W_proj, dtype=np.float32)
    in_maps = []
    for i in range(NCORES):
        b, g = i // 2, i % 2
        j0 = g * JL
        in_maps.append({
            "xT": np.ascontiguousarray(x[b].T).astype(bf16),
            "wq": W_attn[:, j0:j0 + JL].astype(bf16),
            "wk": W_attn[:, E + j0:E + j0 + JL].astype(bf16),
            "wv": W_attn[:, 2 * E + j0:2 * E + j0 + JL].astype(bf16),
            "wp": W_proj[j0:j0 + JL, :].astype(bf16),
            "bq": np.ascontiguousarray(
                b_attn[j0:j0 + JL].astype(np.float32).reshape(JT, P).T),
            "bk": np.ascontiguousarray(
                b_attn[E + j0:E + j0 + JL].astype(np.float32).reshape(JT, P).T),
        })
    return in_maps


def kernel(x, W_attn, b_attn, W_proj, b_proj):
    global _NC_CACHE
    x = np.asarray(x, dtype=np.float32)
    W_attn = np.asarray(W_attn, dtype=np.float32)
    b_attn = np.asarray(b_attn, dtype=np.float32)
    W_proj = np.asarray(W_proj, dtype=np.float32)
    b_proj = np.asarray(b_proj, dtype=np.float32)

    if _NC_CACHE is None:
        _NC_CACHE = build_nc()
    nc = _NC_CACHE

    in_maps = make_in_maps(x, W_attn, b_attn, W_proj, b_proj)
    res = run_bass_kernel_spmd(nc, in_maps, core_ids=list(range(NCORES)))

    # host unshard: sum the two head-group partials + exact bias corrections
    bias_row = b_proj.copy()
    for g in range(2):
        j0 = g * JL
        bv = b_attn[2 * E + j0:2 * E + j0 + JL].astype(np.float32)
        bias_row += bv @ W_proj[j0:j0 + JL, :].astype(np.float32)

    full = np.empty((B, S, E), np.float32)
    for b in range(B):
        full[b] = (res.results[2 * b]["out"].astype(np.float32)
                   + res.results[2 * b + 1]["out"].astype(np.float32)
                   + bias_row[None, :])
    return full


# revision 14
# speedup vs baseline: 1.4956x; 1.0484x over previous
"""Causal multi-head attention block (B=4, S=1024, E=1024, H=16, D=64) on 8 TRN2 cores.

Sharding: data-parallel over batch (4) x tensor-parallel over heads (2 groups of 8).
Core i handles batch i//2, head-group i%2. Each core computes its partial output
projection (row-parallel W_proj); the host sums the two TP partials per batch and
applies the (exact) bias corrections.

Device-side math per core (bf16 compute, f32 accumulate):
  qT = (Wq_g)^T x^T + bq_g          [512, 1024]  (head-major rows h*64+d)
  kT = (Wk_g)^T x^T + bk_g          [512, 1024]
  v  = x Wv_g                       [1024, 512]
  Attention runs in head PAIRS. QK^T is computed per (sk-tile, sq-half) chunk
  with the two heads in the two row-halves of the PE array; both heads' scores
  exp() in ONE strided ScalarE activation into a pair-shared PT tile
  [sk, t, head, sq]. sq-half 0 chunks come first so PV of half 0 overlaps the
  pair's remaining exps. PV + softmax-denominator matmuls are column-tiled
  (concurrent) pairs; a K=1 ones-matmul pair broadcasts the denominators
  across partitions; one reciprocal + one multiply normalizes.
  out_partial = o2T^T Wp_g          [1024, 1024]  (bf16 to DRAM)
Host: out[b] = out_partial[2b] + out_partial[2b+1] + (bv_0 Wp_0 + bv_1 Wp_1 + b_proj)
(the v-bias term is exact because softmax rows sum to 1).
"""

import numpy as np
import ml_dtypes

import concourse.bass as bass
import concourse.tile as tile
from concourse import bacc, mybir
from concourse.bass_utils import run_bass_kernel_spmd
from concourse.masks import make_upper_triangular

BF16 = mybir.dt.bfloat16
F32 = mybir.dt.float32

B, S, E = 4, 1024, 1024
H_TOT, D = 16, 64
NCORES = 8
HL = 8            # heads per core
JL = HL * D       # 512 local qkv dim
P = 128
ET = E // P       # 8 k-tiles over embed dim
JT = JL // P      # 4 partition-tiles over local qkv dim
EXP = mybir.ActivationFunctionType.Exp

_NC_CACHE = None


def build_nc():
    nc = bacc.Bacc()

    xT = nc.declare_dram_parameter("xT", [E, S], BF16, isOutput=False)
    wq = nc.declare_dram_parameter("wq", [E, JL], BF16, isOutput=False)
    wk = nc.declare_dram_parameter("wk", [E, JL], BF16, isOutput=False)
    wv = nc.declare_dram_parameter("wv", [E, JL], BF16, isOutput=False)
    wp = nc.declare_dram_parameter("wp", [JL, E], BF16, isOutput=False)
    bq = nc.declare_dram_parameter("bq", [P, JT], F32, isOutput=False)
    bk = nc.declare_dram_parameter("bk", [P, JT], F32, isOutput=False)
    out = nc.declare_dram_parameter("out", [S, E], BF16, isOutput=True)

    with tile.TileContext(nc) as tc:
        with (
            tc.tile_pool(name="singles", bufs=1) as singles,
            tc.tile_pool(name="pt", bufs=3) as pt_pool,
            tc.tile_pool(name="bc", bufs=2) as bc_pool,
            tc.tile_pool(name="outst", bufs=2) as out_pool,
            tc.tile_pool(name="ps_l", bufs=2, space="PSUM") as ps_l,
            tc.tile_pool(name="ps_mm", bufs=2, space="PSUM") as ps_mm,
            tc.tile_pool(name="ps_o", bufs=2, space="PSUM") as ps_o,
        ):
            # ---- static inputs -> SBUF ----
            xT_sb = singles.tile([P, ET, S], BF16)
            wq_sb = singles.tile([P, ET, JL], BF16)
            wk_sb = singles.tile([P, ET, JL], BF16)
            wv_sb = singles.tile([P, ET, JL], BF16)
            wp_sb = singles.tile([P, JT, E], BF16)
            bq_sb = singles.tile([P, JT], F32)
            bk_sb = singles.tile([P, JT], F32)
            xT_r = xT[:, :].rearrange("(o p) s -> p o s", p=P)
            wq_r = wq[:, :].rearrange("(o p) j -> p o j", p=P)
            wk_r = wk[:, :].rearrange("(o p) j -> p o j", p=P)
            wv_r = wv[:, :].rearrange("(o p) j -> p o j", p=P)
            wp_r = wp[:, :].rearrange("(o p) e -> p o e", p=P)

            # x streams per-ktile on the sync queue; the boot q/k chains
            # (jt0, kt-outer) consume tiles as they land. The scalar queue
            # carries NO input DMAs -- it must be free for the exp table load
            # and the first exps. All weights go on the gpsimd queue as cheap
            # contiguous 2-ktile chunks, wq/wk alternating in boot kt order.
            for kt in range(ET):
                nc.sync.dma_start(out=xT_sb[:, kt], in_=xT_r[:, kt])
            nc.gpsimd.dma_start(out=bq_sb[:], in_=bq[:, :])
            nc.gpsimd.dma_start(out=bk_sb[:], in_=bk[:, :])
            for c in range(0, ET, 2):
                nc.gpsimd.dma_start(out=wq_sb[:, c:c + 2], in_=wq_r[:, c:c + 2])
                nc.gpsimd.dma_start(out=wk_sb[:, c:c + 2], in_=wk_r[:, c:c + 2])
            for c in range(0, ET, 4):
                nc.gpsimd.dma_start(out=wv_sb[:, c:c + 4], in_=wv_r[:, c:c + 4])
            for c in range(0, JT, 2):
                nc.gpsimd.dma_start(out=wp_sb[:, c:c + 2], in_=wp_r[:, c:c + 2])

            # pre-trigger the exp ACT table load (~2.7us) during the DMA wait
            warm_in = singles.tile([1, 1], F32)
            warm_out = singles.tile([1, 1], F32)
            nc.vector.memset(warm_in[:, :], 0.0)
            nc.scalar.activation(out=warm_out[:, :], in_=warm_in[:, :], func=EXP)

            # causal keep-mask for diagonal PT blocks: 1 where sq >= sk else 0
            mask_sb = singles.tile([P, P], BF16)
            make_upper_triangular(nc, mask_sb[:], val=1.0, diag=True)

            # all-ones stationary: the denominator matmul with this lhsT
            # yields the denominator already broadcast across 64 partitions
            ones64 = singles.tile([P, 64], BF16)
            nc.vector.memset(ones64[:, :], 1.0)

            qT_sb = singles.tile([P, JT, S], BF16)   # row j = h*64+d, head-major
            kT_sb = singles.tile([P, JT, S], BF16)
            o2T_sb = singles.tile([P, JT, S], BF16)  # normalized attn out
            v_sb = singles.tile([P, ET, HL, D], BF16)  # [sk_p, sk_tile, head, d]

            # ---- boot: jt0 q/k chains, kt-OUTER so they ride the x stream ----
            psl_q = ps_l.tile([P, 2, 512], F32, tag="psl", name="boot_q")
            psl_k = ps_l.tile([P, 2, 512], F32, tag="psl", name="boot_k")
            for kt in range(ET):
                for w_sb, pslx in ((wq_sb, psl_q), (wk_sb, psl_k)):
                    for nb in range(2):
                        nc.tensor.matmul(
                            pslx[:, nb, :],
                            lhsT=w_sb[:, kt, 0:P],
                            rhs=xT_sb[:, kt, nb * 512:(nb + 1) * 512],
                            start=(kt == 0), stop=(kt == ET - 1),
                            skip_group_check=True,
                        )
            for pslx, b_sb, dst in ((psl_q, bq_sb, qT_sb), (psl_k, bk_sb, kT_sb)):
                for nb in range(2):
                    nc.vector.tensor_scalar_add(
                        dst[:, 0, nb * 512:(nb + 1) * 512],
                        pslx[:, nb, :],
                        b_sb[:, 0:1],
                    )

            # ---- filler units: small PE work packets (~1us each) that get
            # woven between the exp-gated QK chunks so the PE queue never
            # head-of-line blocks on ScalarE ----
            def chain_units(jt, which):
                w_sb, b_sb, dst = ((wq_sb, bq_sb, qT_sb) if which == "q"
                                   else (wk_sb, bk_sb, kT_sb))
                state = {}

                def step(kt0, kt1, first, last):
                    if first:
                        state["pss"] = [
                            ps_mm.tile([P, 512], F32, tag="mm",
                                       name=f"mm_{which}{jt}_{nb}")
                            for nb in range(2)]
                    for kt in range(kt0, kt1):
                        for nb in range(2):
                            nc.tensor.matmul(
                                state["pss"][nb][:],
                                lhsT=w_sb[:, kt, jt * P:(jt + 1) * P],
                                rhs=xT_sb[:, kt, nb * 512:(nb + 1) * 512],
                                start=(kt == 0), stop=(kt == ET - 1),
                            )
                    if last:
                        for nb in range(2):
                            nc.vector.tensor_scalar_add(
                                dst[:, jt, nb * 512:(nb + 1) * 512],
                                state["pss"][nb][:],
                                b_sb[:, jt:jt + 1],
                            )

                return [lambda k0=k0: step(k0, k0 + 2, k0 == 0, k0 == ET - 2)
                        for k0 in range(0, ET, 2)]

            def v_units(st):
                state = {}

                def step(kt0, kt1, first, last):
                    if first:
                        state["ps"] = ps_mm.tile([P, 512], F32, tag="mm",
                                                 name=f"mmv_{st}")
                    for kt in range(kt0, kt1):
                        nc.tensor.matmul(
                            state["ps"][:],
                            lhsT=xT_sb[:, kt, st * P:(st + 1) * P],
                            rhs=wv_sb[:, kt, :],
                            start=(kt == 0), stop=(kt == ET - 1),
                        )
                    if last:
                        nc.vector.tensor_copy(
                            out=v_sb[:, st],
                            in_=state["ps"][:].rearrange("p (h d) -> p h d", h=HL),
                        )

                return [lambda k0=k0: step(k0, k0 + 4, k0 == 0, k0 == 4)
                        for k0 in range(0, ET, 4)]

            # ---- attention in head pairs ----
            # pair p: heads (2p, 2p+1); PT stored pair-shared [sk, t, head, sq]
            def new_ptp(p):
                return pt_pool.tile([P, ET, 2, S], BF16, tag="pt", name=f"ptp_{p}")

            def emit_qk_half(p, ptp, half, filler):
                # half 0: chunks (t=0..3, sq 0-511); half 1: (t=0..3, sq
                # 512-1023) + (t=4..7, sq lo-1023). Both heads' chunk goes
                # through one strided exp. One filler unit is woven in after
                # each chunk to cover the exp shadow.
                chunks = ([(t, 0) for t in range(4)] if half == 0
                          else [(t, 1) for t in range(ET)])
                for t, cbi in chunks:
                    lo = t * P
                    s0 = max(lo, 512 * cbi)
                    c1 = 512 * (cbi + 1)
                    off = s0 - 512 * cbi
                    psl = ps_l.tile([P, 2, 512], F32, tag="psl",
                                    name=f"psl_{p}_{t}_{cbi}")
                    for hh in range(2):
                        po = hh * 64
                        nc.tensor.matmul(
                            psl[:, hh, off:512],
                            lhsT=kT_sb[po:po + 64, p, lo:lo + P],
                            rhs=qT_sb[po:po + 64, p, s0:c1],
                            start=True, stop=True,
                        )
                    nc.scalar.activation(
                        out=ptp[:, t, :, s0:c1], in_=psl[:, :, off:512],
                        func=EXP, scale=0.125,
                    )
                    if filler:
                        filler.popleft()()
                # mask the diagonal blocks of this half's t range, both heads
                t0 = 0 if half == 0 else 4
                for hh in range(2):
                    diag = bass.AP(
                        tensor=ptp.tensor,
                        offset=ptp.offset + hh * S + t0 * (2 * S + P),
                        ap=[list(ptp.ap[0]), [2 * S + P, 4], [1, P]])
                    nc.vector.tensor_mul(
                        out=diag, in0=diag,
                        in1=mask_sb[:, None, :].to_broadcast([P, 4, P]),
                    )

            def emit_pv(p, ptp, sqb):
                c0, c1 = sqb * 512, (sqb + 1) * 512
                ts = [t for t in range(ET) if t * P < c1]
                pso = ps_o.tile([P, 512], F32, tag="pso", name=f"pso_{p}_{sqb}")
                for i, t in enumerate(ts):
                    s0 = max(t * P, c0)
                    off = s0 - c0
                    for hh in range(2):
                        nc.tensor.matmul(
                            pso[hh * 64:(hh + 1) * 64, off:512],
                            lhsT=v_sb[:, t, 2 * p + hh, :],
                            rhs=ptp[:, t, hh, s0:c1],
                            start=(i == 0), stop=(i == len(ts) - 1),
                            skip_group_check=True,
                        )
                den_bc = ps_o.tile([P, 512], F32, tag="pso", name=f"den_{p}_{sqb}")
                for i, t in enumerate(ts):
                    s0 = max(t * P, c0)
                    off = s0 - c0
                    for hh in range(2):
                        nc.tensor.matmul(
                            den_bc[hh * 64:(hh + 1) * 64, off:512],
                            lhsT=ones64[:, :],
                            rhs=ptp[:, t, hh, s0:c1],
                            start=(i == 0), stop=(i == len(ts) - 1),
                            skip_group_check=True,
                        )
                bc_f = bc_pool.tile([P, 512], F32, tag="bcf", name=f"bcf_{p}_{sqb}")
                nc.vector.reciprocal_approx_fast(out=bc_f[:, :], in_=den_bc[:, :])
                nc.vector.tensor_mul(
                    out=o2T_sb[:, p, c0:c1], in0=pso[:, :], in1=bc_f[:, :],
                )

            def proj_units(st):
                state = {}

                def step(eb):
                    if eb == 0:
                        state["obt"] = out_pool.tile([P, E], BF16, tag="ob",
                                                     name=f"ob_{st}")
                    psf = ps_mm.tile([P, 512], F32, tag="mm", name=f"mmp_{st}_{eb}")
                    for kt in range(JT):
                        nc.tensor.matmul(
                            psf[:],
                            lhsT=o2T_sb[:, kt, st * P:(st + 1) * P],
                            rhs=wp_sb[:, kt, eb * 512:(eb + 1) * 512],
                            start=(kt == 0), stop=(kt == JT - 1),
                        )
                    if (st + eb) % 2 == 0:
                        nc.vector.tensor_copy(
                            out=state["obt"][:, eb * 512:(eb + 1) * 512],
                            in_=psf[:])
                    else:
                        nc.scalar.copy(
                            out=state["obt"][:, eb * 512:(eb + 1) * 512],
                            in_=psf[:])
                    if eb == 1:
                        nc.sync.dma_start(out=out[st * P:(st + 1) * P, :],
                                          in_=state["obt"][:])

                return [lambda e=e: step(e) for e in range(2)]

            # ---- master pipeline: per pair, emit QK/exp chunks (sq-half 0
            # first) with one filler unit after each chunk; PV/den of a half
            # as soon as its masks are in; projections interleave into the
            # last pair's half-1 chunks ----
            from collections import deque

            ptps = [new_ptp(0), new_ptp(1)]

            fill = deque()
            fill += v_units(0) + v_units(1) + v_units(2) + v_units(3)
            fill += chain_units(1, "q")
            emit_qk_half(0, ptps[0], 0, fill)        # 4 chunks: v0 v0 v1 v1
            emit_qk_half(0, ptps[0], 1, fill)        # 8: v2 v2 v3 v3 c1q x4
            fill += chain_units(1, "k")
            while fill:
                fill.popleft()()                     # c1k before pair 1
            emit_pv(0, ptps[0], 0)
            fill += v_units(4) + v_units(5) + v_units(6) + v_units(7)
            fill += chain_units(2, "q") + chain_units(2, "k")
            emit_qk_half(1, ptps[1], 0, fill)        # v4 v4 v5 v5
            emit_pv(1, ptps[1], 0)
            emit_qk_half(1, ptps[1], 1, fill)        # v6 v6 v7 v7 c2q x4
            emit_pv(0, ptps[0], 1)
            while fill:
                fill.popleft()()                     # c2k before pair 2
            emit_pv(1, ptps[1], 1)
            ptps.append(new_ptp(2))
            fill += chain_units(3, "q") + chain_units(3, "k")
            emit_qk_half(2, ptps[2], 0, fill)        # c3q x4
            emit_pv(2, ptps[2], 0)
            emit_qk_half(2, ptps[2], 1, fill)        # c3k x4 + 4 empty
            ptps.append(new_ptp(3))
            fill += [lambda: emit_pv(2, ptps[2], 1)]
            emit_qk_half(3, ptps[3], 0, fill)
            while fill:
                fill.popleft()()
            emit_pv(3, ptps[3], 0)
            fill += proj_units(0) + proj_units(1) + proj_units(2) + proj_units(3)
            emit_qk_half(3, ptps[3], 1, fill)        # proj 0-3 interleave
            while fill:
                fill.popleft()()
            emit_pv(3, ptps[3], 1)
            for st in range(4, ET):
                for u in proj_units(st):
                    u()

    nc.compile()
    return nc


def make_in_maps(x, W_attn, b_attn, W_proj, b_proj):
    bf16 = ml_dtypes.bfloat16
    x = np.asarray(x, dtype=np.float32)
    W_attn = np.asarray(W_attn, dtype=np.float32)
    b_attn = np.asarray(b_attn, dtype=np.float32)
    W_proj = np.asarray(W_proj, dtype=np.float32)
    in_maps = []
    for i in range(NCORES):
        b, g = i // 2, i % 2
        j0 = g * JL
        in_maps.append({
            "xT": np.ascontiguousarray(x[b].T).astype(bf16),
            "wq": W_attn[:, j0:j0 + JL].astype(bf16),
            "wk": W_attn[:, E + j0:E + j0 + JL].astype(bf16),
            "wv": W_attn[:, 2 * E + j0:2 * E + j0 + JL].astype(bf16),
            "wp": W_proj[j0:j0 + JL, :].astype(bf16),
            "bq": np.ascontiguousarray(
                b_attn[j0:j0 + JL].astype(np.float32).reshape(JT, P).T),
            "bk": np.ascontiguousarray(
                b_attn[E + j0:E + j0 + JL].astype(np.float32).reshape(JT, P).T),
        })
    return in_maps


def kernel(x, W_attn, b_attn, W_proj, b_proj):
    global _NC_CACHE
    x = np.asarray(x, dtype=np.float32)
    W_attn = np.asarray(W_attn, dtype=np.float32)
    b_attn = np.asarray(b_attn, dtype=np.float32)
    W_proj = np.asarray(W_proj, dtype=np.float32)
    b_proj = np.asarray(b_proj, dtype=np.float32)

    if _NC_CACHE is None:
        _NC_CACHE = build_nc()
    nc = _NC_CACHE

    in_maps = make_in_maps(x, W_attn, b_attn, W_proj, b_proj)
    res = run_bass_kernel_spmd(nc, in_maps, core_ids=list(range(NCORES)))

    # host unshard: sum the two head-group partials + exact bias corrections
    bias_row = b_proj.copy()
    for g in range(2):
        j0 = g * JL
        bv = b_attn[2 * E + j0:2 * E + j0 + JL].astype(np.float32)
        bias_row += bv @ W_proj[j0:j0 + JL, :].astype(np.float32)

    full = np.empty((B, S, E), np.float32)
    for b in range(B):
        full[b] = (res.results[2 * b]["out"].astype(np.float32)
                   + res.results[2 * b + 1]["out"].astype(np.float32)
                   + bias_row[None, :])
    return full
